# revision 1
# baseline (speedup 1.0000x reference)
import os
import sys

import numpy as np

for _p in ("/opt/trn_rl_repo", "/root/.axon_site/_ro/trn_rl_repo"):
    if os.path.isdir(_p) and _p not in sys.path:
        sys.path.insert(0, _p)

# Problem constants (nn_CRF: feats [B,S,T] f32, masks [B,S] ones, transitions [T,T])
B, S, T = 512, 1024, 64
NC = 8            # cores
BL = B // NC      # 64 batches per core
NGRP = 2          # independent batch groups per core (pipelining)
BG = BL // NGRP   # 32
F = 6.0           # upper bound on |feat|; g = exp(feat - F) <= 1
KR = 12           # renormalize every KR g-applications
DBLK = 16         # time steps per DMA block
NBLK = S // DBLK  # 64
NEG = -10000.0

_CACHE = {}


def _build_bass(repeats=None):
    import concourse.bacc as bacc
    import concourse.mybir as mybir
    from concourse.tile import TileContext
    from concourse import bass_isa
    import contextlib

    f32 = mybir.dt.float32
    bf16 = mybir.dt.bfloat16
    Ln = mybir.ActivationFunctionType.Ln

    nc = bacc.Bacc()
    # g arranged host-side as [NBLK, T, DBLK, BL] so each DMA block is
    # contiguous per partition (DBLK*BL*2B = 2KB lines).
    g_in = nc.dram_tensor("g", [NBLK, T, DBLK, BL], bf16, kind="ExternalInput")
    # lhsT for the step matmul: [k, j] = exp(transitions[j, k])
    et_in = nc.dram_tensor("eaug", [T, T], bf16, kind="ExternalInput")
    xout = nc.dram_tensor("xout", [T, BL], bf16, kind="ExternalOutput")
    aux = nc.dram_tensor("aux", [2, BL], f32, kind="ExternalOutput")

    NX = 4      # X state rotation slots (gives gpsimd slack to read old X)
    LAG = 4     # renorm scale measured at app b is folded into g at app b+LAG

    with TileContext(nc) as tc:
        with tc.tile_pool(name="const", bufs=1) as cpool, \
             tc.tile_pool(name="gp", bufs=3) as gpool, \
             tc.tile_pool(name="state", bufs=1) as xpool, \
             tc.tile_pool(name="ps", bufs=1, space="PSUM") as pspool, \
             tc.tile_pool(name="misc", bufs=2) as mpool:
            et_stage = cpool.tile([T, T], bf16)
            nc.sync.dma_start(et_stage, et_in[:, :])
            et = cpool.tile([T, T], bf16)
            # copy via DVE so matmuls depend only on the DVE semaphore
            nc.vector.tensor_copy(et, et_stage)
            loop_cm = tc.For_i(0, repeats, 1) if repeats else contextlib.nullcontext()
            with loop_cm:
                xs, crows, pss, sbs, rbss, gs2s = [], [], [], [], [], []
                for gi in range(NGRP):
                    rot = []
                    for sl_i in range(NX):
                        x_t = xpool.tile([T, BG], bf16, tag=f"x{gi}_{sl_i}")
                        rot.append(x_t)
                    xs.append(rot)
                    cr = xpool.tile([1, BG], f32, tag=f"c{gi}")
                    nc.vector.memset(cr, 0.0)
                    crows.append(cr)
                    ps_t = pspool.tile([T, BG], f32, tag=f"ps{gi}")
                    pss.append(ps_t)
                    sb_t = xpool.tile([T, BG], f32, tag=f"sb{gi}")
                    sbs.append(sb_t)
                    rbs_t = xpool.tile([T, BG], f32, tag=f"rbs{gi}")
                    rbss.append(rbs_t)
                    gs2_t = xpool.tile([T, BG], bf16, tag=f"gs2{gi}")
                    gs2s.append(gs2_t)
                # pending[gi] = app index whose g-slice must be scaled by rbss
                pending = [None] * NGRP
                sb_last = [None] * NGRP
                app = 0
                for blk in range(NBLK):
                    gt = gpool.tile([T, DBLK, BL], bf16, tag="g")
                    nc.sync.dma_start(gt, g_in[blk])
                    for t in range(DBLK):
                        for gi in range(NGRP):
                            gsl = gt[:, t, gi * BG:(gi + 1) * BG]
                            if app == 0:
                                nc.vector.tensor_copy(xs[gi][0], gsl)
                                continue
                            xprev = xs[gi][(app - 1) % NX]
                            xcur = xs[gi][app % NX]
                            ps = pss[gi]
                            nc.tensor.matmul(ps, et, xprev, start=True, stop=True)
                            if pending[gi] == app:
                                # fold the pending 1/s renorm into this g slice
                                # (off the critical chain: all-SBUF DVE ops)
                                nc.vector.tensor_mul(gs2s[gi], gsl, rbss[gi])
                                gsl = gs2s[gi]
                                pending[gi] = None
                            nc.vector.tensor_mul(xcur, gsl, ps)
                            is_tap = (app % KR == KR - 1 and app + LAG <= S - 2)
                            if is_tap or app == S - 2:
                                # partition sum of X_app, broadcast to all
                                # partitions; gpsimd runs off the chain
                                nc.gpsimd.partition_all_reduce(
                                    sbs[gi], xcur, T, bass_isa.ReduceOp.add)
                                ls = mpool.tile([1, BG], f32, tag=f"l{gi}")
                                nc.scalar.activation(ls, sbs[gi][0:1, :], Ln)
                                if is_tap:
                                    nc.vector.reciprocal(rbss[gi], sbs[gi])
                                    nc.vector.tensor_add(crows[gi], crows[gi], ls)
                                    pending[gi] = app + LAG
                                if app == S - 2:
                                    sb_last[gi] = ls
                        app += 1
                for gi in range(NGRP):
                    cs = slice(gi * BG, (gi + 1) * BG)
                    nc.sync.dma_start(xout[:, cs], xs[gi][(S - 1) % NX])
                    nc.sync.dma_start(aux[0:1, cs], crows[gi])
                    nc.sync.dma_start(aux[1:2, cs], sb_last[gi])
    nc.finalize()
    return nc


def _numpy_ref(feats, masks, transitions):
    # Exact log-domain fallback (only used if masks are not all ones).
    alpha = feats[:, 0].astype(np.float64)
    tr = transitions.astype(np.float64)
    for i in range(1, feats.shape[1]):
        sc = alpha[:, None, :] + tr[None] + feats[:, i, :, None].astype(np.float64)
        m = sc.max(axis=2, keepdims=True)
        new = (m[:, :, 0] + np.log(np.exp(sc - m).sum(axis=2)))
        mask = masks[:, i, None].astype(np.float64)
        alpha = new * mask + alpha * (1.0 - mask)
    return alpha.astype(np.float32)


def kernel(feats, masks, transitions):
    feats = np.asarray(feats, dtype=np.float32)
    masks = np.asarray(masks, dtype=np.float32)
    transitions = np.asarray(transitions, dtype=np.float32)
    if not np.all(masks == 1.0):
        return _numpy_ref(feats, masks, transitions)

    from concourse import bass_utils

    if "nc" not in _CACHE:
        _CACHE["nc"] = _build_bass()
    nc = _CACHE["nc"]

    E = np.exp(transitions)                      # [j,k]; row/col 0 -> 0
    # overflow-safety: per-step growth bound must fit f32 over a KR+4 window
    grow = float(np.log(E.sum(axis=1)).max())
    assert (KR + 4) * max(grow, 0.0) < 85.0, grow
    eaug = np.ascontiguousarray(E.T)
    # g[b,s,j] = exp(feats - F) -> per-core [NBLK, T, DBLK, BL] bf16
    g = np.exp(feats - F)
    g = g.reshape(NC, BL, NBLK, DBLK, T).transpose(0, 2, 4, 3, 1)
    g = np.ascontiguousarray(g, dtype=np.float32)
    import ml_dtypes
    g16 = g.astype(ml_dtypes.bfloat16)

    in_maps = [{"g": g16[c], "eaug": eaug.astype(ml_dtypes.bfloat16)} for c in range(NC)]
    trace = bool(os.environ.get("CRF_TRACE"))
    import time as _time
    _t0 = _time.time()
    res = bass_utils.run_bass_kernel_spmd(
        nc, in_maps, core_ids=list(range(NC)), trace=trace)
    _CACHE.setdefault("t_run", []).append(_time.time() - _t0)
    _CACHE["last_res"] = res

    alpha = np.empty((B, T), np.float32)
    for c in range(NC):
        X = res.results[c]["xout"].astype(np.float64)    # [T, BL]
        crow = res.results[c]["aux"][0].astype(np.float64)   # [BL]
        lsl = res.results[c]["aux"][1].astype(np.float64)    # ln sum_k X_{S-2}
        a = np.log(np.maximum(X.T, 1e-300)) + (S * F + crow)[:, None]
        a[:, 0] = (feats[c * BL:(c + 1) * BL, S - 1, 0] + NEG
                   + lsl + (S - 1) * F + crow)
        alpha[c * BL:(c + 1) * BL] = a.astype(np.float32)
    return alpha



# revision 3
# speedup vs baseline: 1.2625x; 1.2625x over previous
import os
import sys

import numpy as np

for _p in ("/opt/trn_rl_repo", "/root/.axon_site/_ro/trn_rl_repo"):
    if os.path.isdir(_p) and _p not in sys.path:
        sys.path.insert(0, _p)

# Problem constants (nn_CRF: feats [B,S,T] f32, masks [B,S] ones, transitions [T,T])
B, S, T = 512, 1024, 64
NC = 8            # cores
BL = B // NC      # 64 batches per core
NCH = 2           # independent chains per core (pipelining PE <-> DVE)
FREE = 16         # batch columns per instruction (per chain: 2 partition halves x FREE)
DBLK = 32         # time steps per DMA block
NBLK = S // DBLK  # 32
NEG = -10000.0

_CACHE = {}


def _build_bass():
    import concourse.bacc as bacc
    import concourse.mybir as mybir
    from concourse.tile import TileContext

    f32 = mybir.dt.float32
    bf16 = mybir.dt.bfloat16

    nc = bacc.Bacc()
    # g arranged host-side as [NBLK, 128, DBLK, NCH, FREE] so each DMA block
    # is contiguous per partition (DBLK*NCH*FREE*2B = 2KB lines).
    g_in = nc.dram_tensor("g", [NBLK, 128, DBLK, NCH, FREE], bf16, kind="ExternalInput")
    # Stationary operand: blockdiag(E^T, E^T) [128, 128];
    # out[64h+j, f] = sum_k E[j,k] X[64h+k, f]
    w_in = nc.dram_tensor("eaug", [128, 128], bf16, kind="ExternalInput")
    xout = nc.dram_tensor("xout", [128, NCH * FREE], bf16, kind="ExternalOutput")
    # X at step S-2, for the column-0 reconstruction (needs ln sum_k X_{S-2,k})
    xout2 = nc.dram_tensor("xout2", [128, NCH * FREE], bf16, kind="ExternalOutput")

    NX = 4      # X state rotation slots

    with TileContext(nc) as tc:
        with tc.tile_pool(name="const", bufs=1) as cpool, \
             tc.tile_pool(name="gp", bufs=3) as gpool, \
             tc.tile_pool(name="state", bufs=1) as xpool, \
             tc.tile_pool(name="ps", bufs=1, space="PSUM") as pspool:
            w_stage = cpool.tile([128, 128], bf16)
            nc.sync.dma_start(w_stage, w_in[:, :])
            w = cpool.tile([128, 128], bf16)
            # copy via DVE so matmuls depend only on the DVE semaphore
            nc.vector.tensor_copy(w, w_stage)
            xs, pss = [], []
            for ci in range(NCH):
                rot = [xpool.tile([128, FREE], bf16, tag=f"x{ci}_{sl}",
                                  name=f"x{ci}_{sl}") for sl in range(NX)]
                xs.append(rot)
                pss.append(pspool.tile([128, FREE], f32, tag=f"ps{ci}",
                                       name=f"ps{ci}"))
            app = 0
            for blk in range(NBLK):
                gt = gpool.tile([128, DBLK, NCH, FREE], bf16, tag="g")
                nc.sync.dma_start(gt, g_in[blk])
                for t in range(DBLK):
                    for ci in range(NCH):
                        gsl = gt[:, t, ci, :]
                        if app == 0:
                            nc.vector.tensor_copy(xs[ci][0], gsl)
                            continue
                        xprev = xs[ci][(app - 1) % NX]
                        xcur = xs[ci][app % NX]
                        ps = pss[ci]
                        nc.tensor.matmul(ps, w, xprev, start=True, stop=True)
                        nc.vector.tensor_mul(xcur, gsl, ps)
                    app += 1
            for ci in range(NCH):
                cs = slice(ci * FREE, (ci + 1) * FREE)
                nc.sync.dma_start(xout[:, cs], xs[ci][(S - 1) % NX])
                nc.sync.dma_start(xout2[:, cs], xs[ci][(S - 2) % NX])
    nc.finalize()
    return nc


def _numpy_ref(feats, masks, transitions):
    # Exact log-domain fallback (only used if masks are not all ones).
    alpha = feats[:, 0].astype(np.float64)
    tr = transitions.astype(np.float64)
    for i in range(1, feats.shape[1]):
        sc = alpha[:, None, :] + tr[None] + feats[:, i, :, None].astype(np.float64)
        m = sc.max(axis=2, keepdims=True)
        new = (m[:, :, 0] + np.log(np.exp(sc - m).sum(axis=2)))
        mask = masks[:, i, None].astype(np.float64)
        alpha = new * mask + alpha * (1.0 - mask)
    return alpha.astype(np.float32)


def _estimate_F(feats, E):
    """Mean per-step log-growth of sum(X), sampled over a few batches in f64.

    Sets F so the renorm-free recursion X_{s+1} = exp(feat-F) * (E X_s) has
    zero mean exponent drift; also returns the max |wander| seen so the
    caller can verify the bf16/f32 exponent budget.
    """
    idx = np.linspace(0, feats.shape[0] - 1, 16).astype(int)
    Et = E.T.astype(np.float64)
    X = np.exp(feats[idx, 0].astype(np.float64))
    X /= X.sum(1, keepdims=True)
    cum = np.zeros(len(idx))
    cums = [cum.copy()]
    for s in range(1, feats.shape[1]):
        X = np.exp(feats[idx, s].astype(np.float64)) * (X @ Et)
        sm = X.sum(1)
        X /= sm[:, None]
        cum = cum + np.log(sm)
        cums.append(cum.copy())
    cums = np.stack(cums)                       # [S, nb] cumulative ln-growth
    F = float(cum.mean()) / (feats.shape[1] - 1)
    drift = cums - F * np.arange(cums.shape[0])[:, None]
    wander = float(np.abs(drift).max())
    return F, wander


def kernel(feats, masks, transitions):
    feats = np.asarray(feats, dtype=np.float32)
    masks = np.asarray(masks, dtype=np.float32)
    transitions = np.asarray(transitions, dtype=np.float32)
    if not np.all(masks == 1.0):
        return _numpy_ref(feats, masks, transitions)

    from concourse import bass_utils

    if "nc" not in _CACHE:
        _CACHE["nc"] = _build_bass()
    nc = _CACHE["nc"]

    E = np.exp(transitions)                      # [j,k]; row/col 0 -> 0
    F, wander = _estimate_F(feats, E)
    # exponent budget: sampled wander + cross-batch spread + profile depth
    # must stay well inside bf16/f32 range (|ln X| < ~85)
    if not (wander + 25.0 < 55.0):
        return _numpy_ref(feats, masks, transitions)

    W = np.zeros((128, 128), np.float32)
    W[:64, :64] = E.T
    W[64:, 64:] = E.T
    import ml_dtypes
    # g[b,s,j] = exp(feats - F) -> per-core [NBLK, 128, DBLK, NCH, FREE] bf16
    g = np.exp(feats - F)
    # [NC, NCH(c), 2(h), FREE(f), NBLK, DBLK, T(j)] -> [NC, NBLK, h*64+j, DBLK, c, f]
    g = g.reshape(NC, NCH, 2, FREE, NBLK, DBLK, T).transpose(0, 4, 2, 6, 5, 1, 3)
    g = np.ascontiguousarray(g.reshape(NC, NBLK, 128, DBLK, NCH, FREE), dtype=np.float32)
    g16 = g.astype(ml_dtypes.bfloat16)

    w16 = W.astype(ml_dtypes.bfloat16)
    in_maps = [{"g": g16[c], "eaug": w16} for c in range(NC)]
    trace = bool(os.environ.get("CRF_TRACE"))
    res = bass_utils.run_bass_kernel_spmd(
        nc, in_maps, core_ids=list(range(NC)), trace=trace)
    _CACHE["last_res"] = res

    alpha = np.empty((B, T), np.float32)
    for c in range(NC):
        X1 = res.results[c]["xout"].astype(np.float64)    # [128, NCH*FREE]
        X2 = res.results[c]["xout2"].astype(np.float64)
        # [p=64h+j, 16c+f] -> b_local = 32c+16h+f
        X1 = X1.reshape(2, T, NCH, FREE).transpose(2, 0, 3, 1).reshape(BL, T)
        X2 = X2.reshape(2, T, NCH, FREE).transpose(2, 0, 3, 1).reshape(BL, T)
        a = np.log(np.maximum(X1, 1e-300)) + S * F
        lsl = np.log(np.maximum(X2.sum(axis=1), 1e-300))  # ln sum_k X_{S-2,k}
        a[:, 0] = (feats[c * BL:(c + 1) * BL, S - 1, 0] + NEG
                   + lsl + (S - 1) * F)
        alpha[c * BL:(c + 1) * BL] = a.astype(np.float32)
    return alpha


# revision 4
# speedup vs baseline: 3.2273x; 2.5563x over previous
import os
import sys

import numpy as np

for _p in ("/opt/trn_rl_repo", "/root/.axon_site/_ro/trn_rl_repo"):
    if os.path.isdir(_p) and _p not in sys.path:
        sys.path.insert(0, _p)

# nn_CRF: feats [B,S,T] f32, masks [B,S] ones, transitions [T,T].
# Renorm-free exp-domain recursion X_{s+1} = exp(feat-F) * (E X_s), run as
# Q time-chunks per core IN PARALLEL: products of positive matrices contract
# to rank-1 (Birkhoff), so chunk i>0 starts from an arbitrary positive vector
# W warm-up steps before its range and is glued to chunk i-1 host-side by a
# per-batch scalar ratio at the handoff step.  This breaks the per-step
# MM->TT->MM latency chain (433ns) that bounds a single serial recursion.
B, S, T = 512, 1024, 64
NC = 8            # cores
BL = B // NC      # 64 batches per core
Q = 4             # parallel time-chunks per core
WARM = 32         # warm-up iterations for chunks 1..Q-1 (snapshot at iter WARM-1)
L_IT = 280        # iterations per chunk (1 init copy + L_IT-1 steps)
STARTS = (0, 248, 496, 744)   # chunk i state starts at step STARTS[i]
DBLK = 8          # iterations per DMA block
NBLK = L_IT // DBLK  # 35
NEG = -10000.0

_CACHE = {}


def _build_bass():
    import concourse.bacc as bacc
    import concourse.mybir as mybir
    from concourse.tile import TileContext

    f32 = mybir.dt.float32
    bf16 = mybir.dt.bfloat16

    nc = bacc.Bacc()
    # g arranged host-side as [NBLK, 128, DBLK, Q, 32]: per DMA block each
    # partition gets DBLK*Q*32*2B = 2KB contiguous.
    g_in = nc.dram_tensor("g", [NBLK, 128, DBLK, Q, 32], bf16, kind="ExternalInput")
    # Stationary operand: blockdiag(E^T, E^T) [128, 128];
    # out[64h+j, m] = sum_k E[j,k] X[64h+k, m]
    w_in = nc.dram_tensor("eaug", [128, 128], bf16, kind="ExternalInput")
    # slots 0..2: warm snaps of chunks 1..3; 3..5: finals of chunks 0..2;
    # 6: chunk3 state at S-2; 7: chunk3 final (S-1)
    xall = nc.dram_tensor("xall", [8, 128, 32], bf16, kind="ExternalOutput")

    NX = 4      # X state rotation slots per chunk

    with TileContext(nc) as tc:
        with tc.tile_pool(name="const", bufs=1) as cpool, \
             tc.tile_pool(name="gp", bufs=3) as gpool, \
             tc.tile_pool(name="state", bufs=1) as xpool, \
             tc.tile_pool(name="ps", bufs=1, space="PSUM") as pspool:
            w_stage = cpool.tile([128, 128], bf16)
            nc.sync.dma_start(w_stage, w_in[:, :])
            w = cpool.tile([128, 128], bf16)
            # copy via DVE so matmuls depend only on the DVE semaphore
            nc.vector.tensor_copy(w, w_stage)
            xs, pss, snaps = [], [], []
            for ci in range(Q):
                rot = [xpool.tile([128, 32], bf16, tag=f"x{ci}_{sl}",
                                  name=f"x{ci}_{sl}") for sl in range(NX)]
                xs.append(rot)
                pss.append(pspool.tile([128, 32], f32, tag=f"ps{ci}",
                                       name=f"ps{ci}"))
                if ci > 0:
                    snaps.append(xpool.tile([128, 32], bf16, tag=f"sn{ci}",
                                            name=f"sn{ci}"))
            for blk in range(NBLK):
                gt = gpool.tile([128, DBLK, Q, 32], bf16, tag="g")
                nc.sync.dma_start(gt, g_in[blk])
                for t in range(DBLK):
                    k = blk * DBLK + t
                    for ci in range(Q):
                        gsl = gt[:, t, ci, :]
                        if k == 0:
                            nc.vector.tensor_copy(xs[ci][0], gsl)
                            continue
                        xprev = xs[ci][(k - 1) % NX]
                        xcur = xs[ci][k % NX]
                        nc.tensor.matmul(pss[ci], w, xprev, start=True, stop=True)
                        nc.vector.tensor_mul(xcur, gsl, pss[ci])
                        if ci > 0 and k == WARM - 1:
                            nc.vector.tensor_copy(snaps[ci - 1], xcur)
            for i in range(Q - 1):
                nc.sync.dma_start(xall[i], snaps[i])
                nc.sync.dma_start(xall[3 + i], xs[i][(L_IT - 1) % NX])
            nc.sync.dma_start(xall[6], xs[Q - 1][(L_IT - 2) % NX])
            nc.sync.dma_start(xall[7], xs[Q - 1][(L_IT - 1) % NX])
    nc.finalize()
    return nc


def _numpy_ref(feats, masks, transitions):
    # Exact log-domain fallback (only used if masks are not all ones or the
    # fast path's safety checks trip).
    alpha = feats[:, 0].astype(np.float64)
    tr = transitions.astype(np.float64)
    for i in range(1, feats.shape[1]):
        sc = alpha[:, None, :] + tr[None] + feats[:, i, :, None].astype(np.float64)
        m = sc.max(axis=2, keepdims=True)
        new = (m[:, :, 0] + np.log(np.exp(sc - m).sum(axis=2)))
        mask = masks[:, i, None].astype(np.float64)
        alpha = new * mask + alpha * (1.0 - mask)
    return alpha.astype(np.float32)


def _estimate_F(feats, E):
    """Mean per-step log-growth of sum(X), sampled over a few batches in f64.

    Sets F so the renorm-free recursion has ~zero mean exponent drift, and
    returns the max |wander| seen so the caller can verify the exponent
    budget."""
    idx = np.linspace(0, feats.shape[0] - 1, 16).astype(int)
    Et = E.T.astype(np.float64)
    X = np.exp(feats[idx, 0].astype(np.float64))
    X /= X.sum(1, keepdims=True)
    cum = np.zeros(len(idx))
    cums = [cum.copy()]
    for s in range(1, feats.shape[1]):
        X = np.exp(feats[idx, s].astype(np.float64)) * (X @ Et)
        sm = X.sum(1)
        X /= sm[:, None]
        cum = cum + np.log(sm)
        cums.append(cum.copy())
    cums = np.stack(cums)
    F = float(cum.mean()) / (feats.shape[1] - 1)
    drift = cums - F * np.arange(cums.shape[0])[:, None]
    return F, float(np.abs(drift).max())


def _unpack(X):
    # [p=64h+j, m] -> [b_l = 32h+m, j]
    return X.reshape(2, T, 32).transpose(0, 2, 1).reshape(BL, T)


def kernel(feats, masks, transitions):
    feats = np.asarray(feats, dtype=np.float32)
    masks = np.asarray(masks, dtype=np.float32)
    transitions = np.asarray(transitions, dtype=np.float32)
    if not np.all(masks == 1.0):
        return _numpy_ref(feats, masks, transitions)

    from concourse import bass_utils

    if "nc" not in _CACHE:
        _CACHE["nc"] = _build_bass()
    nc = _CACHE["nc"]

    E = np.exp(transitions)                      # [j,k]; row/col 0 -> 0
    F, wander = _estimate_F(feats, E)
    if not (wander < 40.0):
        return _numpy_ref(feats, masks, transitions)

    Wmat = np.zeros((128, 128), np.float32)
    Wmat[:64, :64] = E.T
    Wmat[64:, 64:] = E.T
    import ml_dtypes
    g = np.exp(feats - F)
    # packed per core: G[core, s, p=64h+j, m] = g[b=core*64+32h+m, s, j]
    G = g.reshape(NC, 2, 32, S, T).transpose(0, 3, 1, 4, 2).reshape(NC, S, 128, 32)
    idx = (np.asarray(STARTS)[None, :] + np.arange(L_IT)[:, None])  # [L_IT, Q]
    g_hw = G[:, idx]                             # [NC, L_IT, Q, 128, 32]
    g_hw = g_hw.reshape(NC, NBLK, DBLK, Q, 128, 32).transpose(0, 1, 4, 2, 3, 5)
    g_hw = np.ascontiguousarray(g_hw, dtype=np.float32).astype(ml_dtypes.bfloat16)

    w16 = Wmat.astype(ml_dtypes.bfloat16)
    in_maps = [{"g": g_hw[c], "eaug": w16} for c in range(NC)]
    trace = bool(os.environ.get("CRF_TRACE"))
    res = bass_utils.run_bass_kernel_spmd(
        nc, in_maps, core_ids=list(range(NC)), trace=trace)
    _CACHE["last_res"] = res

    alpha = np.empty((B, T), np.float32)
    ok = True
    for c in range(NC):
        xa = res.results[c]["xall"].astype(np.float64)   # [8, 128, 32]
        sl = [_unpack(xa[i]) for i in range(8)]
        lnr = np.zeros(BL)
        for i in range(Q - 1):
            snap, fin = sl[i], sl[3 + i]
            r = snap.sum(1) / np.maximum(fin.sum(1), 1e-300)
            if not np.all(r > 0):
                ok = False
            lnr += np.log(np.maximum(r, 1e-300))
        zfin, zpen = sl[7], sl[6]
        a = np.log(np.maximum(zfin, 1e-300)) + S * F - lnr[:, None]
        lsl = np.log(np.maximum(zpen.sum(1), 1e-300)) - lnr
        a[:, 0] = (feats[c * BL:(c + 1) * BL, S - 1, 0] + NEG
                   + lsl + (S - 1) * F)
        alpha[c * BL:(c + 1) * BL] = a.astype(np.float32)
    if not ok or not np.all(np.isfinite(alpha)):
        return _numpy_ref(feats, masks, transitions)
    return alpha


# revision 6
# speedup vs baseline: 7.7621x; 2.4051x over previous
import os
import sys

import numpy as np

for _p in ("/opt/trn_rl_repo", "/root/.axon_site/_ro/trn_rl_repo"):
    if os.path.isdir(_p) and _p not in sys.path:
        sys.path.insert(0, _p)

# nn_CRF: feats [B,S,T] f32, masks [B,S] ones, transitions [T,T].
#
# Renorm-free exp-domain recursion X_{s+1} = exp(feat_{s+1}-F) * (E X_s).
# Products of positive matrices contract to rank-1 (Birkhoff), so the
# recursion forgets its start in ~10 steps; we run NQ=24 overlapping
# time-chunks per core IN PARALLEL (chunk 0 from the true X_0, chunk i from
# an arbitrary positive start ~14 steps before its range) and glue scales
# host-side with one per-batch scalar ratio per handoff.  Chunks are packed
# 8-wide into the free dim so each chain-iteration is ONE [128,128]x[128,256]
# matmul + ONE [128,256] DVE multiply; 3 such chains hide the per-step
# MM->sem->TT->sem round-trip (~650ns) while the DVE stays saturated.
B, S, T = 512, 1024, 64
NC = 8            # cores
BL = B // NC      # 64 batches per core
NQ = 24           # time-chunks per core
MW = 8            # chunks merged per chain
NCHAIN = NQ // MW  # 3
L_IT = 57         # iterations per chain (1 init copy + 56 steps)
# chunk i ends (= handoff point of chunk i+1) at step H[i]; starts at ST[i].
H = [L_IT - 1] + [L_IT - 1 + (967 * i) // 23 for i in range(1, NQ)]
ST = [h - (L_IT - 1) for h in H]
# chunk i>=1 passes its predecessor's end at iteration KSNAP[i] (13 or 14)
KSNAP = [None] + [H[i - 1] - ST[i] for i in range(1, NQ)]
DBLK = 3          # iterations per DMA block
NBLK = L_IT // DBLK  # 19
NEG = -10000.0

_CACHE = {}


def _build_bass():
    import concourse.bacc as bacc
    import concourse.mybir as mybir
    from concourse.tile import TileContext

    f32 = mybir.dt.float32
    bf16 = mybir.dt.bfloat16

    nc = bacc.Bacc()
    # g arranged host-side as [NBLK, 128, DBLK, NCHAIN, 256]: per DMA block
    # each partition gets DBLK*NCHAIN*256*2B = 4.5KB contiguous.
    g_in = nc.dram_tensor("g", [NBLK, 128, DBLK, NCHAIN, MW * 32], bf16,
                          kind="ExternalInput")
    # Stationary operand: blockdiag(E^T, E^T) [128, 128]
    w_in = nc.dram_tensor("eaug", [128, 128], bf16, kind="ExternalInput")
    # whole-chain snapshots at iterations 13 and 14 (host picks per chunk)
    snapA = nc.dram_tensor("snapA", [NCHAIN, 128, MW * 32], bf16,
                           kind="ExternalOutput")
    snapB = nc.dram_tensor("snapB", [NCHAIN, 128, MW * 32], bf16,
                           kind="ExternalOutput")
    xfin = nc.dram_tensor("xfin", [NCHAIN, 128, MW * 32], bf16,
                          kind="ExternalOutput")
    xpen = nc.dram_tensor("xpen", [128, MW * 32], bf16, kind="ExternalOutput")

    NX = 8      # X state rotation slots per chain

    with TileContext(nc) as tc:
        with tc.tile_pool(name="const", bufs=1) as cpool, \
             tc.tile_pool(name="gp", bufs=3) as gpool, \
             tc.tile_pool(name="state", bufs=1) as xpool, \
             tc.tile_pool(name="ps", bufs=1, space="PSUM") as pspool:
            w_stage = cpool.tile([128, 128], bf16)
            nc.sync.dma_start(w_stage, w_in[:, :])
            w = cpool.tile([128, 128], bf16)
            # copy via DVE so matmuls depend only on the DVE semaphore
            nc.vector.tensor_copy(w, w_stage)
            xs, pss = [], []
            for ci in range(NCHAIN):
                rot = [xpool.tile([128, MW * 32], bf16, tag=f"x{ci}_{sl}",
                                  name=f"x{ci}_{sl}") for sl in range(NX)]
                xs.append(rot)
                pss.append(pspool.tile([128, MW * 32], f32, tag=f"ps{ci}",
                                       name=f"ps{ci}"))
            for blk in range(NBLK):
                gt = gpool.tile([128, DBLK, NCHAIN, MW * 32], bf16, tag="g")
                nc.sync.dma_start(gt, g_in[blk])
                for t in range(DBLK):
                    k = blk * DBLK + t
                    for ci in range(NCHAIN):
                        gsl = gt[:, t, ci, :]
                        if k == 0:
                            nc.vector.tensor_copy(xs[ci][0], gsl)
                            continue
                        xprev = xs[ci][(k - 1) % NX]
                        xcur = xs[ci][k % NX]
                        nc.tensor.matmul(pss[ci], w, xprev, start=True, stop=True)
                        nc.vector.tensor_mul(xcur, gsl, pss[ci])
                        if k == 13:
                            nc.sync.dma_start(snapA[ci], xcur)
                        elif k == 14:
                            nc.sync.dma_start(snapB[ci], xcur)
            for ci in range(NCHAIN):
                nc.sync.dma_start(xfin[ci], xs[ci][(L_IT - 1) % NX])
            nc.sync.dma_start(xpen[:, :], xs[NCHAIN - 1][(L_IT - 2) % NX])
    nc.finalize()
    return nc


def _numpy_ref(feats, masks, transitions):
    # Exact log-domain fallback (only used if masks are not all ones or the
    # fast path's safety checks trip).
    alpha = feats[:, 0].astype(np.float64)
    tr = transitions.astype(np.float64)
    for i in range(1, feats.shape[1]):
        sc = alpha[:, None, :] + tr[None] + feats[:, i, :, None].astype(np.float64)
        m = sc.max(axis=2, keepdims=True)
        new = (m[:, :, 0] + np.log(np.exp(sc - m).sum(axis=2)))
        mask = masks[:, i, None].astype(np.float64)
        alpha = new * mask + alpha * (1.0 - mask)
    return alpha.astype(np.float32)


def _estimate_F(feats, E):
    """Mean per-step log-growth of sum(X), sampled over a few batches in f64."""
    idx = np.linspace(0, feats.shape[0] - 1, 16).astype(int)
    Et = E.T.astype(np.float64)
    X = np.exp(feats[idx, 0].astype(np.float64))
    X /= X.sum(1, keepdims=True)
    cum = np.zeros(len(idx))
    cums = [cum.copy()]
    for s in range(1, feats.shape[1]):
        X = np.exp(feats[idx, s].astype(np.float64)) * (X @ Et)
        sm = X.sum(1)
        X /= sm[:, None]
        cum = cum + np.log(sm)
        cums.append(cum.copy())
    cums = np.stack(cums)
    F = float(cum.mean()) / (feats.shape[1] - 1)
    drift = cums - F * np.arange(cums.shape[0])[:, None]
    return F, float(np.abs(drift).max())


def _unpack(X):
    # [p=64h+j, m] -> [b_l = 32h+m, j]
    return X.reshape(2, T, 32).transpose(0, 2, 1).reshape(BL, T)


def kernel(feats, masks, transitions):
    feats = np.asarray(feats, dtype=np.float32)
    masks = np.asarray(masks, dtype=np.float32)
    transitions = np.asarray(transitions, dtype=np.float32)
    if not np.all(masks == 1.0):
        return _numpy_ref(feats, masks, transitions)

    from concourse import bass_utils

    if "nc" not in _CACHE:
        _CACHE["nc"] = _build_bass()
    nc = _CACHE["nc"]

    E = np.exp(transitions)                      # [j,k]; row/col 0 -> 0
    F, wander = _estimate_F(feats, E)
    if not (wander < 40.0):
        return _numpy_ref(feats, masks, transitions)

    Wmat = np.zeros((128, 128), np.float32)
    Wmat[:64, :64] = E.T
    Wmat[64:, 64:] = E.T
    import ml_dtypes
    g = np.exp(feats - F)
    # packed per core: G[core, s, p=64h+j, m] = g[b=core*64+32h+m, s, j]
    G = g.reshape(NC, 2, 32, S, T).transpose(0, 3, 1, 4, 2).reshape(NC, S, 128, 32)
    idx = (np.asarray(ST)[None, :] + np.arange(L_IT)[:, None])   # [L_IT, NQ]
    g_hw = G[:, idx]                             # [NC, L_IT, NQ, 128, 32]
    g_hw = g_hw.reshape(NC, L_IT, NCHAIN, MW, 128, 32).transpose(0, 1, 2, 4, 3, 5)
    g_hw = g_hw.reshape(NC, NBLK, DBLK, NCHAIN, 128, MW * 32)
    g_hw = np.ascontiguousarray(g_hw.transpose(0, 1, 4, 2, 3, 5), dtype=np.float32)
    g_hw = g_hw.astype(ml_dtypes.bfloat16)

    w16 = Wmat.astype(ml_dtypes.bfloat16)
    in_maps = [{"g": g_hw[c], "eaug": w16} for c in range(NC)]
    trace = bool(os.environ.get("CRF_TRACE"))
    res = bass_utils.run_bass_kernel_spmd(
        nc, in_maps, core_ids=list(range(NC)), trace=trace)
    _CACHE["last_res"] = res

    alpha = np.empty((B, T), np.float32)
    ok = True
    for c in range(NC):
        r = res.results[c]
        snA = r["snapA"].astype(np.float64)      # [NCHAIN, 128, 256]
        snB = r["snapB"].astype(np.float64)
        fin = r["xfin"].astype(np.float64)
        pen = r["xpen"].astype(np.float64)

        def chunk_slice(arr, i):
            return _unpack(arr[i // MW][:, 32 * (i % MW):32 * (i % MW) + 32])

        lnr = np.zeros(BL)
        for i in range(1, NQ):
            sn = snA if KSNAP[i] == 13 else snB
            snap_i = chunk_slice(sn, i)
            fin_prev = chunk_slice(fin, i - 1)
            rr = snap_i.sum(1) / np.maximum(fin_prev.sum(1), 1e-300)
            if not np.all(rr > 0):
                ok = False
            lnr += np.log(np.maximum(rr, 1e-300))
        zfin = chunk_slice(fin, NQ - 1)
        zpen = _unpack(pen[:, 32 * (MW - 1):])
        a = np.log(np.maximum(zfin, 1e-300)) + S * F - lnr[:, None]
        lsl = np.log(np.maximum(zpen.sum(1), 1e-300)) - lnr
        a[:, 0] = (feats[c * BL:(c + 1) * BL, S - 1, 0] + NEG
                   + lsl + (S - 1) * F)
        alpha[c * BL:(c + 1) * BL] = a.astype(np.float32)
    if not ok or not np.all(np.isfinite(alpha)):
        return _numpy_ref(feats, masks, transitions)
    return alpha


# revision 7
# speedup vs baseline: 8.5205x; 1.0977x over previous
import os
import sys

import numpy as np

for _p in ("/opt/trn_rl_repo", "/root/.axon_site/_ro/trn_rl_repo"):
    if os.path.isdir(_p) and _p not in sys.path:
        sys.path.insert(0, _p)

# nn_CRF: feats [B,S,T] f32, masks [B,S] ones, transitions [T,T].
#
# Renorm-free exp-domain recursion X_{s+1} = exp(feat_{s+1}-F) * (E X_s).
# Products of positive matrices contract to rank-1 (Birkhoff), so the
# recursion forgets its start in ~10 steps; we run NQ=24 overlapping
# time-chunks per core IN PARALLEL (chunk 0 from the true X_0, chunk i from
# an arbitrary positive start ~14 steps before its range) and glue scales
# host-side with one per-batch scalar ratio per handoff.  Chunks are packed
# 8-wide into the free dim so each chain-iteration is ONE [128,128]x[128,256]
# matmul + ONE [128,256] DVE multiply; 3 such chains hide the per-step
# MM->sem->TT->sem round-trip (~650ns) while the DVE stays saturated.
B, S, T = 512, 1024, 64
NC = 8            # cores
BL = B // NC      # 64 batches per core
NQ = 24           # time-chunks per core
MW = 8            # chunks merged per chain
NCHAIN = NQ // MW  # 3
L_IT = 51         # iterations per chain (1 init copy + 50 steps)
# chunk i ends (= handoff point of chunk i+1) at step H[i]; starts at ST[i].
H = [L_IT - 1] + [L_IT - 1 + ((1023 - (L_IT - 1)) * i) // (NQ - 1)
     for i in range(1, NQ)]
ST = [h - (L_IT - 1) for h in H]
# chunk i>=1 passes its predecessor's end at iteration KSNAP[i] (7 or 8)
KSNAP = [None] + [H[i - 1] - ST[i] for i in range(1, NQ)]
KSNAP_LO = min(KSNAP[1:])
assert set(KSNAP[1:]) <= {KSNAP_LO, KSNAP_LO + 1}
DBLK = 3          # iterations per DMA block
NBLK = L_IT // DBLK  # 17
NEG = -10000.0

_CACHE = {}


def _build_bass():
    import concourse.bacc as bacc
    import concourse.mybir as mybir
    from concourse.tile import TileContext

    f32 = mybir.dt.float32
    bf16 = mybir.dt.bfloat16

    nc = bacc.Bacc()
    # g arranged host-side as [NBLK, 128, DBLK, NCHAIN, 256]: per DMA block
    # each partition gets DBLK*NCHAIN*256*2B = 4.5KB contiguous.
    g_in = nc.dram_tensor("g", [NBLK, 128, DBLK, NCHAIN, MW * 32], bf16,
                          kind="ExternalInput")
    # Stationary operand: blockdiag(E^T, E^T) [128, 128]
    w_in = nc.dram_tensor("eaug", [128, 128], bf16, kind="ExternalInput")
    # whole-chain snapshots at the two handoff iterations (host picks per chunk)
    snapA = nc.dram_tensor("snapA", [NCHAIN, 128, MW * 32], bf16,
                           kind="ExternalOutput")
    snapB = nc.dram_tensor("snapB", [NCHAIN, 128, MW * 32], bf16,
                           kind="ExternalOutput")
    xfin = nc.dram_tensor("xfin", [NCHAIN, 128, MW * 32], bf16,
                          kind="ExternalOutput")
    xpen = nc.dram_tensor("xpen", [128, MW * 32], bf16, kind="ExternalOutput")

    NX = 8      # X state rotation slots per chain

    with TileContext(nc) as tc:
        with tc.tile_pool(name="const", bufs=1) as cpool, \
             tc.tile_pool(name="gp", bufs=4) as gpool, \
             tc.tile_pool(name="state", bufs=1) as xpool, \
             tc.tile_pool(name="ps", bufs=1, space="PSUM") as pspool:
            w_stage = cpool.tile([128, 128], bf16)
            nc.sync.dma_start(w_stage, w_in[:, :])
            w = cpool.tile([128, 128], bf16)
            # copy via DVE so matmuls depend only on the DVE semaphore
            nc.vector.tensor_copy(w, w_stage)
            xs, pss = [], []
            for ci in range(NCHAIN):
                rot = [xpool.tile([128, MW * 32], bf16, tag=f"x{ci}_{sl}",
                                  name=f"x{ci}_{sl}") for sl in range(NX)]
                xs.append(rot)
                pss.append(pspool.tile([128, MW * 32], f32, tag=f"ps{ci}",
                                       name=f"ps{ci}"))
            for blk in range(NBLK):
                gt = gpool.tile([128, DBLK, NCHAIN, MW * 32], bf16, tag="g")
                nc.sync.dma_start(gt, g_in[blk])
                for t in range(DBLK):
                    k = blk * DBLK + t
                    for ci in range(NCHAIN):
                        gsl = gt[:, t, ci, :]
                        if k == 0:
                            nc.vector.tensor_copy(xs[ci][0], gsl)
                            continue
                        xprev = xs[ci][(k - 1) % NX]
                        xcur = xs[ci][k % NX]
                        nc.tensor.matmul(pss[ci], w, xprev, start=True, stop=True)
                        nc.vector.tensor_mul(xcur, gsl, pss[ci])
                        if k == KSNAP_LO:
                            nc.sync.dma_start(snapA[ci], xcur)
                        elif k == KSNAP_LO + 1:
                            nc.sync.dma_start(snapB[ci], xcur)
            for ci in range(NCHAIN):
                nc.sync.dma_start(xfin[ci], xs[ci][(L_IT - 1) % NX])
            nc.sync.dma_start(xpen[:, :], xs[NCHAIN - 1][(L_IT - 2) % NX])
    nc.finalize()
    return nc


def _numpy_ref(feats, masks, transitions):
    # Exact log-domain fallback (only used if masks are not all ones or the
    # fast path's safety checks trip).
    alpha = feats[:, 0].astype(np.float64)
    tr = transitions.astype(np.float64)
    for i in range(1, feats.shape[1]):
        sc = alpha[:, None, :] + tr[None] + feats[:, i, :, None].astype(np.float64)
        m = sc.max(axis=2, keepdims=True)
        new = (m[:, :, 0] + np.log(np.exp(sc - m).sum(axis=2)))
        mask = masks[:, i, None].astype(np.float64)
        alpha = new * mask + alpha * (1.0 - mask)
    return alpha.astype(np.float32)


def _estimate_F(feats, E):
    """Mean per-step log-growth of sum(X), sampled over a few batches in f64."""
    idx = np.linspace(0, feats.shape[0] - 1, 16).astype(int)
    Et = E.T.astype(np.float64)
    X = np.exp(feats[idx, 0].astype(np.float64))
    X /= X.sum(1, keepdims=True)
    cum = np.zeros(len(idx))
    cums = [cum.copy()]
    for s in range(1, feats.shape[1]):
        X = np.exp(feats[idx, s].astype(np.float64)) * (X @ Et)
        sm = X.sum(1)
        X /= sm[:, None]
        cum = cum + np.log(sm)
        cums.append(cum.copy())
    cums = np.stack(cums)
    F = float(cum.mean()) / (feats.shape[1] - 1)
    drift = cums - F * np.arange(cums.shape[0])[:, None]
    return F, float(np.abs(drift).max())


def _unpack(X):
    # [p=64h+j, m] -> [b_l = 32h+m, j]
    return X.reshape(2, T, 32).transpose(0, 2, 1).reshape(BL, T)


def kernel(feats, masks, transitions):
    feats = np.asarray(feats, dtype=np.float32)
    masks = np.asarray(masks, dtype=np.float32)
    transitions = np.asarray(transitions, dtype=np.float32)
    if not np.all(masks == 1.0):
        return _numpy_ref(feats, masks, transitions)

    from concourse import bass_utils

    if "nc" not in _CACHE:
        _CACHE["nc"] = _build_bass()
    nc = _CACHE["nc"]

    E = np.exp(transitions)                      # [j,k]; row/col 0 -> 0
    F, wander = _estimate_F(feats, E)
    if not (wander < 40.0):
        return _numpy_ref(feats, masks, transitions)

    Wmat = np.zeros((128, 128), np.float32)
    Wmat[:64, :64] = E.T
    Wmat[64:, 64:] = E.T
    import ml_dtypes
    g = np.exp(feats - F)
    # packed per core: G[core, s, p=64h+j, m] = g[b=core*64+32h+m, s, j]
    G = g.reshape(NC, 2, 32, S, T).transpose(0, 3, 1, 4, 2).reshape(NC, S, 128, 32)
    idx = (np.asarray(ST)[None, :] + np.arange(L_IT)[:, None])   # [L_IT, NQ]
    g_hw = G[:, idx]                             # [NC, L_IT, NQ, 128, 32]
    g_hw = g_hw.reshape(NC, L_IT, NCHAIN, MW, 128, 32).transpose(0, 1, 2, 4, 3, 5)
    g_hw = g_hw.reshape(NC, NBLK, DBLK, NCHAIN, 128, MW * 32)
    g_hw = np.ascontiguousarray(g_hw.transpose(0, 1, 4, 2, 3, 5), dtype=np.float32)
    g_hw = g_hw.astype(ml_dtypes.bfloat16)

    w16 = Wmat.astype(ml_dtypes.bfloat16)
    in_maps = [{"g": g_hw[c], "eaug": w16} for c in range(NC)]
    trace = bool(os.environ.get("CRF_TRACE"))
    res = bass_utils.run_bass_kernel_spmd(
        nc, in_maps, core_ids=list(range(NC)), trace=trace)
    _CACHE["last_res"] = res

    alpha = np.empty((B, T), np.float32)
    ok = True
    for c in range(NC):
        r = res.results[c]
        snA = r["snapA"].astype(np.float64)      # [NCHAIN, 128, 256]
        snB = r["snapB"].astype(np.float64)
        fin = r["xfin"].astype(np.float64)
        pen = r["xpen"].astype(np.float64)

        def chunk_slice(arr, i):
            return _unpack(arr[i // MW][:, 32 * (i % MW):32 * (i % MW) + 32])

        lnr = np.zeros(BL)
        for i in range(1, NQ):
            sn = snA if KSNAP[i] == KSNAP_LO else snB
            snap_i = chunk_slice(sn, i)
            fin_prev = chunk_slice(fin, i - 1)
            rr = snap_i.sum(1) / np.maximum(fin_prev.sum(1), 1e-300)
            if not np.all(rr > 0):
                ok = False
            lnr += np.log(np.maximum(rr, 1e-300))
        zfin = chunk_slice(fin, NQ - 1)
        zpen = _unpack(pen[:, 32 * (MW - 1):])
        a = np.log(np.maximum(zfin, 1e-300)) + S * F - lnr[:, None]
        lsl = np.log(np.maximum(zpen.sum(1), 1e-300)) - lnr
        a[:, 0] = (feats[c * BL:(c + 1) * BL, S - 1, 0] + NEG
                   + lsl + (S - 1) * F)
        alpha[c * BL:(c + 1) * BL] = a.astype(np.float32)
    if not ok or not np.all(np.isfinite(alpha)):
        return _numpy_ref(feats, masks, transitions)
    return alpha
